# revision 3
# baseline (speedup 1.0000x reference)
"""Trainium2 Bass kernel: batched multi-head self-attention (nn_Attention).

y = softmax(q k^T / sqrt(64)) v, projected; x (8, 1025, 768), 12 heads x 64.

Strategy: batch-parallel across the 8 NeuronCores (one batch element per
core, no collectives). Per core, everything is feature-major (transposed).

Queries are processed in THREE equal 344-column chunks (3 x 344 = 1032
padded tokens), so there is no 8-wide tail-query pass: the v1 design's
(512, 512, 8) chunking spent ~47us of PE time on ~320 8-column matmuls
whose cost is pure per-instruction overhead. All matmuls now stream
>=344 columns.

Per chunk, the 6 head-pairs run as software-pipelined attention blocks:
score matmuls for the two heads of a pair run CONCURRENTLY in the PE
array (64x128 row tiling, h0 rows 0-63 / h1 rows 64-127, distinct PSUM
banks: h0 cols 0:344 of bank A, h1 cols 512:856 = bank B), one strided
exp per key tile covers both heads ([ksz, 2x344] grouped access
pattern). QKV projection, V layout build, and output projection are
woven as filler matmuls between the attention steps so the PE never
idles behind a projection phase and the scalar engine (exp) stays fed.
Softmax denominators ride as a ones-column in the V operand; chunks 0/1
normalize batched (12-head reciprocal + selector-matmul broadcast),
chunk 2 normalizes per-pair so the kernel tail is short.

Operands fp16, exp'd weights bf16, accumulation fp32 in PSUM.
"""
import sys

try:
    import concourse.bass  # noqa: F401
except ImportError:
    sys.path.insert(0, "/opt/trn_rl_repo")

import numpy as np

from contextlib import ExitStack

import concourse.bass as bass
import concourse.tile as tile
from concourse import bacc, mybir

F32 = mybir.dt.float32
BF16 = mybir.dt.bfloat16
F16 = mybir.dt.float16

C = 768
H = 12
D = 64
NTOK = 1025
T = 1032
CT = C // 128
SCALE = D ** -0.5

KT = [(i * 128, 128) for i in range(8)] + [(1024, 8)]
QW = 344
QC = [(0, QW), (QW, QW), (2 * QW, QW)]
VC = [(0, 512), (512, 256)]
VW = 65


def build(matmul_dtype="fp16"):
    MT = AT = F16
    ATTN = BF16
    nc = bacc.Bacc("TRN2", target_bir_lowering=False, debug=False, num_devices=8)

    xT_d = nc.dram_tensor("xT", [C, T], MT, kind="ExternalInput")
    wqkT_d = nc.dram_tensor("wqkT", [C, 2 * C], MT, kind="ExternalInput")
    wvT_d = nc.dram_tensor("wvT", [C, C], MT, kind="ExternalInput")
    wpT_d = nc.dram_tensor("wpT", [C, C], MT, kind="ExternalInput")
    bp_d = nc.dram_tensor("bp", [C, 1], F32, kind="ExternalInput")
    sel_d = nc.dram_tensor("sel", [128, 768], F16, kind="ExternalInput")
    yT_d = nc.dram_tensor("yT", [C, T], F16, kind="ExternalOutput")

    with tile.TileContext(nc) as tc, ExitStack() as ctx:
        p_x = ctx.enter_context(tc.tile_pool(name="x", bufs=1))
        p_w = ctx.enter_context(tc.tile_pool(name="w", bufs=1))
        p_qk = ctx.enter_context(tc.tile_pool(name="qk", bufs=1))
        p_v = ctx.enter_context(tc.tile_pool(name="v", bufs=1))
        p_ao = ctx.enter_context(tc.tile_pool(name="ao", bufs=1))
        p_at = ctx.enter_context(tc.tile_pool(name="at", bufs=1))
        p_sm = ctx.enter_context(tc.tile_pool(name="sm", bufs=1))
        p_stage = ctx.enter_context(tc.tile_pool(name="stage", bufs=1))
        ps_sc = ctx.enter_context(tc.tile_pool(name="psc", bufs=2, space="PSUM"))
        ps_av = ctx.enter_context(tc.tile_pool(name="psav", bufs=2, space="PSUM"))
        ps_pr = ctx.enter_context(tc.tile_pool(name="pspr", bufs=2, space="PSUM"))

        xT = [p_x.tile([128, T], MT, tag=f"x{i}", name=f"x{i}") for i in range(CT)]
        wqk = [p_w.tile([128, 2 * C], MT, tag=f"wqk{i}", name=f"wqk{i}")
               for i in range(CT)]
        wv = [p_w.tile([128, C], MT, tag=f"wv{i}", name=f"wv{i}") for i in range(CT)]
        wp = [p_w.tile([128, C], MT, tag=f"wp{i}", name=f"wp{i}") for i in range(CT)]
        bp_sb = [p_w.tile([128, 1], F32, tag=f"bp{i}", name=f"bp{i}")
                 for i in range(CT)]
        qkT = [p_qk.tile([128, T], AT, tag=f"qkT{i}", name=f"qkT{i}")
               for i in range(12)]
        v_ext = [p_v.tile([128, H * VW + 63], AT, tag=f"v{i}", name=f"v{i}")
                 for i in range(9)]
        aoT = [p_ao.tile([128, T], MT, tag=f"ao{i}", name=f"ao{i}")
               for i in range(CT)]

        # minimal slices for qk pair 0 first so the attention pipeline can
        # start ~2us in, then the rest
        for c in range(CT):
            nc.sync.dma_start(xT[c][:, 0:QW], xT_d.ap()[c * 128:(c + 1) * 128, 0:QW])
        for c in range(CT):
            nc.sync.dma_start(wqk[c][:, 0:128], wqkT_d.ap()[c * 128:(c + 1) * 128, 0:128])
            nc.sync.dma_start(wqk[c][:, 768:896],
                              wqkT_d.ap()[c * 128:(c + 1) * 128, 768:896])
        for c in range(CT):
            nc.sync.dma_start(xT[c][:, QW:T], xT_d.ap()[c * 128:(c + 1) * 128, QW:T])
        for c in range(CT):
            nc.sync.dma_start(wv[c][:], wvT_d.ap()[c * 128:(c + 1) * 128, :])
        for c in range(CT):
            nc.sync.dma_start(wqk[c][:, 128:768],
                              wqkT_d.ap()[c * 128:(c + 1) * 128, 128:768])
            nc.sync.dma_start(wqk[c][:, 896:1536],
                              wqkT_d.ap()[c * 128:(c + 1) * 128, 896:1536])
        for c in range(CT):
            nc.sync.dma_start(wp[c][:], wpT_d.ap()[c * 128:(c + 1) * 128, :])
            nc.sync.dma_start(bp_sb[c][:], bp_d.ap()[c * 128:(c + 1) * 128, :])

        # selector for the PE-based denominator-reciprocal broadcast
        # (host-built constant): sel[2p, 128p:128p+64] = 1,
        # sel[2p+1, 128p+64:128p+128] = 1, so sel[:, 128p:128(p+1)].T @ rec12
        # lands head 2p's reciprocal on psum rows 0:64, head 2p+1's on 64:128
        # zero-padded to K=128 so the broadcast matmuls stay full-array
        # (partial-K matmuls trip the PE clock governor to half speed).
        # sel[0:2, 0:128] doubles as the K=2 per-pair selector.
        sel = p_w.tile([128, 768], F16, tag="sel", name="sel")
        nc.sync.dma_start(sel[:], sel_d.ap()[:, :])

        # ---------- filler units (each emits ~one PSUM chunk of proj work) ----

        def qk_unit(ot, qoff, qsz):
            def go():
                ps = ps_pr.tile([128, 512], F32, tag="pr", name="ps_pr")
                for c in range(CT):
                    nc.tensor.matmul(
                        ps[:, :qsz],
                        wqk[c][:, ot * 128:(ot + 1) * 128],
                        xT[c][:, qoff:qoff + qsz],
                        start=(c == 0), stop=(c == CT - 1),
                    )
                nc.vector.tensor_copy(qkT[ot][:, qoff:qoff + qsz], ps[:, :qsz])
            return go

        def qk_req_units(p):
            # needed before attn_block(p, chunk 0): q chunk 0, k all chunks
            order = [(p, QC[0]), (6 + p, QC[0]), (6 + p, QC[1]), (6 + p, QC[2])]
            return [qk_unit(ot, qoff, qsz) for ot, (qoff, qsz) in order]

        def q_unit(p, ci):
            return qk_unit(p, *QC[ci])

        def v_unit(nt, voff, vsz):
            noff, nsz = KT[nt]

            def go():
                ps = ps_pr.tile([128, 512], F32, tag="pr", name="ps_pr")
                for c in range(CT):
                    nc.tensor.matmul(
                        ps[:nsz, :vsz],
                        xT[c][:, noff:noff + nsz],
                        wv[c][:, voff:voff + vsz],
                        start=(c == 0), stop=(c == CT - 1),
                    )
                if nt == 8:
                    # duplicate the 8 tail-key v rows at psum rows 32:40 so
                    # h1's tail AV (weights at partitions 32:40) is
                    # partition-aligned with its stationary operand
                    for c in range(CT):
                        nc.tensor.matmul(
                            ps[32:32 + nsz, :vsz],
                            xT[c][:, noff:noff + nsz],
                            wv[c][:, voff:voff + vsz],
                            start=(c == 0), stop=(c == CT - 1),
                        )
                nh = vsz // D
                h0 = voff // D

                def vcopy(r0):
                    dst = (
                        v_ext[nt][r0:r0 + nsz, h0 * VW:(h0 + nh) * VW]
                        .rearrange("p (hh w) -> p hh w", w=VW)[:, :, 0:D]
                    )
                    src = ps[r0:r0 + nsz, 0:vsz].rearrange("p (hh w) -> p hh w", w=D)
                    nc.vector.tensor_copy(dst, src)

                vcopy(0)
                if nt == 8:
                    vcopy(32)
                if voff + vsz == C:
                    # ones column (valid tokens only) + zeroed pad/tail
                    if nt < 8:
                        ones_col = (
                            v_ext[nt][0:nsz, 0:H * VW]
                            .rearrange("p (hh w) -> p hh w", w=VW)[:, :, D:VW]
                        )
                        _memset(nc, AT, ones_col, one=True)
                    else:
                        for r0 in (0, 32):
                            pad_col = (
                                v_ext[nt][r0:r0 + nsz, 0:H * VW]
                                .rearrange("p (hh w) -> p hh w", w=VW)[:, :, D:VW]
                            )
                            _memset(nc, AT, pad_col, one=False)
                            one_row = (
                                v_ext[nt][r0:r0 + 1, 0:H * VW]
                                .rearrange("p (hh w) -> p hh w", w=VW)[:, :, D:VW]
                            )
                            _memset(nc, AT, one_row, one=True)
                    _memset(nc, AT, v_ext[nt][:, H * VW:H * VW + 63], one=False)
            return go

        def v_units():
            return [v_unit(nt, voff, vsz) for nt in range(9) for (voff, vsz) in VC]

        def e_unit(ot, qoff, qsz):
            def go():
                ps = ps_pr.tile([128, 512], F32, tag="pr", name="ps_pr")
                for c in range(CT):
                    nc.tensor.matmul(
                        ps[:, :qsz],
                        wp[c][:, ot * 128:(ot + 1) * 128],
                        aoT[c][:, qoff:qoff + qsz],
                        start=(c == 0), stop=(c == CT - 1),
                    )
                st = p_stage.tile([128, 512], F16, tag="ystage", name="ystage",
                                  bufs=4)
                nc.vector.tensor_scalar_add(st[:, :qsz], ps[:, :qsz],
                                            bp_sb[ot][:, 0:1])
                nc.sync.dma_start(
                    yT_d.ap()[ot * 128:(ot + 1) * 128, qoff:qoff + qsz],
                    st[:, :qsz])
            return go

        # ---------- softmax normalization ------------------------------------

        # all 12 heads' av (64 rows) + denominator (row 64) per chunk, stashed
        # from PSUM right after each pair's AV so the PSUM banks free quickly
        def make_chunk_state():
            return {"av": p_sm.tile([VW, 12 * QW], F32, tag="avall",
                                    name="avall", bufs=2)}

        def stash_av(cs, av_ps, h, qsz):
            nc.vector.tensor_copy(cs["av"][0:VW, h * QW:h * QW + qsz],
                                  av_ps[0:VW, 0:qsz])

        # batched (chunks 0/1): one DMA gathers the denominator row into
        # [12, QW], one reciprocal, then a full-array fp16 selector matmul
        # broadcasts each pair's reciprocals across PSUM partitions
        def chunk_normalize(cs, qoff, qsz):
            rec16 = p_sm.tile([128, 512], F16, tag="rec16", name="rec16", bufs=2)
            den12 = p_sm.tile([12, 512], F32, tag="den12", name="den12", bufs=2)
            nc.sync.dma_start(den12[0:12, 0:qsz], cs["av"][D:VW, 0:12 * qsz])
            rec12 = p_sm.tile([12, 512], F32, tag="rec12", name="rec12", bufs=2)
            nc.vector.reciprocal(rec12[0:12, 0:qsz], den12[0:12, 0:qsz])
            nc.vector.memset(rec16[:].bitcast(mybir.dt.uint16), 0)
            nc.vector.tensor_copy(rec16[0:12, 0:qsz], rec12[0:12, 0:qsz])
            for pair in range(6):
                bc_ps = ps_pr.tile([128, 512], F32, tag="pr", name="ps_pr")
                nc.tensor.matmul(
                    bc_ps[0:128, 0:qsz],
                    sel[0:128, 128 * pair:128 * (pair + 1)],
                    rec16[0:128, 0:qsz],
                    start=True, stop=True,
                )
                for hi in range(2):
                    h = 2 * pair + hi
                    nc.vector.tensor_mul(
                        aoT[pair][64 * hi:64 * hi + 64, qoff:qoff + qsz],
                        cs["av"][0:D, h * QW:h * QW + qsz],
                        bc_ps[64 * hi:64 * hi + 64, 0:qsz],
                    )

        # per-pair (chunk 2): normalize pair p right after its block so only
        # pair 5's short chain + output projection remain after the last block
        def pair_normalize(cs, pair, qoff, qsz):
            den2 = p_sm.tile([2, 512], F32, tag="den2", name="den2", bufs=3)
            nc.sync.dma_start(den2[0:2, 0:qsz],
                              cs["av"][D:VW, 2 * pair * QW:(2 * pair + 2) * QW])
            rec2 = p_sm.tile([2, 512], F32, tag="rec2", name="rec2", bufs=3)
            nc.vector.reciprocal(rec2[0:2, 0:qsz], den2[0:2, 0:qsz])
            rec2h = p_sm.tile([2, 512], F16, tag="rec2h", name="rec2h", bufs=3)
            nc.vector.tensor_copy(rec2h[0:2, 0:qsz], rec2[0:2, 0:qsz])
            bc_ps = ps_pr.tile([128, 512], F32, tag="pr", name="ps_pr")
            nc.tensor.matmul(
                bc_ps[0:128, 0:qsz],
                sel[0:2, 0:128],
                rec2h[0:2, 0:qsz],
                start=True, stop=True,
            )
            for hi in range(2):
                h = 2 * pair + hi
                nc.vector.tensor_mul(
                    aoT[pair][64 * hi:64 * hi + 64, qoff:qoff + qsz],
                    cs["av"][0:D, h * QW:h * QW + qsz],
                    bc_ps[64 * hi:64 * hi + 64, 0:qsz],
                )

        # ---------- attention block for one head pair, one query chunk -------

        def attn_block(pair, qoff, qsz, fillers, fill_per_kt, cs, warm=False):
            h0, h1 = 2 * pair, 2 * pair + 1
            fill_iter = iter(fillers)

            def fill(n):
                for _ in range(n):
                    f = next(fill_iter, None)
                    if f is not None:
                        f()

            avs = {
                h0: ps_av.tile([128, 512], F32, tag="av", name="ps_av"),
                h1: ps_av.tile([128, 512], F32, tag="av", name="ps_av"),
            }
            pend = []  # (kt, at_tile) awaiting AV emission

            def scores_kt(kt, warm=False):
                koff, ksz = KT[kt]
                sc = ps_sc.tile([128, 1024], F32, tag="sc", name="ps_sc")
                if warm:
                    # full-array dummy matmul holds the PE clock governor at
                    # 8/8 through the half-array score runs (overwritten by
                    # the real scores below)
                    nc.tensor.matmul(
                        sc[0:128, 0:512],
                        qkT[0][0:128, 0:128],
                        qkT[0][0:128, 0:512],
                        start=True, stop=True,
                    )
                # the two heads' score matmuls run concurrently (row groups
                # 0/64) into one psum tile: h0 at cols 0:QW (bank 0), h1 at
                # 512:512+QW (bank 1)
                nc.tensor.matmul(
                    sc[0:ksz, 0:qsz],
                    qkT[6 + pair][0:64, koff:koff + ksz],
                    qkT[pair][0:64, qoff:qoff + qsz],
                    start=True, stop=True,
                )
                nc.tensor.matmul(
                    sc[0:ksz, 512:512 + qsz],
                    qkT[6 + pair][64:128, koff:koff + ksz],
                    qkT[pair][64:128, qoff:qoff + qsz],
                    start=True, stop=True,
                )
                # one strided exp covers both heads: [ksz, 2 x qsz] groups at
                # cols {0, 512} in, packed [ksz, 2*qsz] out
                at = p_at.tile([128, 2 * QW], ATTN, tag="attnT", name="attnT",
                               bufs=6)
                nc.scalar.activation(
                    at[0:ksz, 0:2 * qsz].rearrange("p (b w) -> p b w", w=qsz),
                    sc[0:ksz, 0:1024].rearrange("p (b w) -> p b w", w=512)[:, :, 0:qsz],
                    mybir.ActivationFunctionType.Exp, scale=SCALE,
                )
                return (kt, at)

            def drain_av():
                while pend:
                    pkt, pat = pend.pop(0)
                    _, pksz = KT[pkt]
                    for hi, h in enumerate((h0, h1)):
                        nc.tensor.matmul(
                            avs[h][0:128, 0:qsz],
                            v_ext[pkt][0:pksz, h * VW:h * VW + 128],
                            pat[0:pksz, hi * qsz:hi * qsz + qsz],
                            start=(pkt == 0), stop=False,
                            skip_group_check=True,
                        )

            # 2-kt bursts keep the PE in 64-row tiling mode across 4 score
            # matmuls before switching to 128-row AV/filler matmuls; AV
            # emission lags one burst so its exp input is ready
            for kb in range(4):
                new = [scores_kt(2 * kb, warm=warm), scores_kt(2 * kb + 1)]
                fill(fill_per_kt)
                drain_av()
                pend.extend(new)
                fill(fill_per_kt)

            # tail key tile (8 keys): h0 scores at psum rows 0:8, h1 at 32:40
            # (col-group aligned with the v_ext[8] stationary rows)
            koff, ksz = KT[8]
            sc8 = ps_pr.tile([128, 512], F32, tag="pr", name="ps_pr")
            nc.tensor.matmul(
                sc8[0:8, 0:qsz],
                qkT[6 + pair][0:64, koff:koff + ksz],
                qkT[pair][0:64, qoff:qoff + qsz],
                start=True, stop=True,
            )
            nc.tensor.matmul(
                sc8[32:40, 0:qsz],
                qkT[6 + pair][64:128, koff:koff + ksz],
                qkT[pair][64:128, qoff:qoff + qsz],
                start=True, stop=True,
            )
            at8 = p_at.tile([128, 2 * QW], ATTN, tag="attnT", name="attnT", bufs=6)
            nc.scalar.activation(
                at8[0:8, 0:qsz], sc8[0:8, 0:qsz],
                mybir.ActivationFunctionType.Exp, scale=SCALE,
            )
            nc.scalar.activation(
                at8[32:40, 0:qsz], sc8[32:40, 0:qsz],
                mybir.ActivationFunctionType.Exp, scale=SCALE,
            )
            # drain pending AV (kt 6/7), then the tail AV closes accumulation
            while pend:
                pkt, pat = pend.pop(0)
                _, pksz = KT[pkt]
                for hi, h in enumerate((h0, h1)):
                    nc.tensor.matmul(
                        avs[h][0:128, 0:qsz],
                        v_ext[pkt][0:pksz, h * VW:h * VW + 128],
                        pat[0:pksz, hi * qsz:hi * qsz + qsz],
                        start=False, stop=False,
                        skip_group_check=True,
                    )
            for hi, h in enumerate((h0, h1)):
                nc.tensor.matmul(
                    avs[h][0:128, 0:qsz],
                    v_ext[8][32 * hi:32 * hi + 8, h * VW:h * VW + 128],
                    at8[32 * hi:32 * hi + 8, 0:qsz],
                    start=False, stop=True,
                    skip_group_check=True,
                )
            stash_av(cs, avs[h0], h0, qsz)
            stash_av(cs, avs[h1], h1, qsz)
            fill(99)  # drain leftovers

        # ---------- schedule ---------------------------------------------------

        with nc.named_scope("pass0"):
            cs0 = make_chunk_state()
            for u in qk_req_units(0)[:2]:
                u()  # q(0,c0), k(0,c0): enough for burst 0 of block 0
            deferred_k0 = qk_req_units(0)[2:]
            p0_fillers = {
                0: deferred_k0 + v_units() + qk_req_units(1),
                1: qk_req_units(2) + [q_unit(0, 1), q_unit(1, 1)],
                2: qk_req_units(3) + [q_unit(2, 1), q_unit(3, 1)],
                3: qk_req_units(4) + [q_unit(4, 1), q_unit(5, 1)],
                4: qk_req_units(5) + [q_unit(0, 2), q_unit(1, 2)],
                5: [q_unit(2, 2), q_unit(3, 2), q_unit(4, 2), q_unit(5, 2)],
            }
            for p in range(6):
                attn_block(p, QC[0][0], QC[0][1], p0_fillers[p],
                           fill_per_kt=(3 if p == 0 else 1), cs=cs0)
            chunk_normalize(cs0, QC[0][0], QC[0][1])

        def e_fillers(ci):
            # chunk-ci output projection rides the block ends (fill(99)),
            # shifted one block so normalize(ci) has a full block of slack
            # before its first consumer
            return {0: [], 1: [e_unit(0, *QC[ci])], 2: [e_unit(1, *QC[ci])],
                    3: [e_unit(2, *QC[ci])], 4: [e_unit(3, *QC[ci])],
                    5: [e_unit(4, *QC[ci]), e_unit(5, *QC[ci])]}

        with nc.named_scope("pass1"):
            cs1 = make_chunk_state()
            f1 = e_fillers(0)
            for p in range(6):
                attn_block(p, QC[1][0], QC[1][1], f1[p],
                           fill_per_kt=0, cs=cs1, warm=True)
            chunk_normalize(cs1, QC[1][0], QC[1][1])

        with nc.named_scope("pass2"):
            cs2 = make_chunk_state()
            f2 = e_fillers(1)
            for p in range(6):
                attn_block(p, QC[2][0], QC[2][1], f2[p],
                           fill_per_kt=0, cs=cs2, warm=True)
                pair_normalize(cs2, p, QC[2][0], QC[2][1])
            for ot in range(CT):
                e_unit(ot, *QC[2])()

    nc.compile()
    return nc


def _memset(nc, AT, ap, one):
    if AT == BF16:
        nc.vector.memset(ap.bitcast(mybir.dt.uint16), 0x3F80 if one else 0)
    elif AT == F16:
        nc.vector.memset(ap.bitcast(mybir.dt.uint16), 0x3C00 if one else 0)
    else:
        nc.vector.memset(ap.bitcast(mybir.dt.uint32), 0x3F800000 if one else 0)


_NC_CACHE = {}
_MODE = "fp16"


def prep_in_maps(x, w_qkv, w_proj, b_proj, mode=None):
    mode = mode or _MODE
    x = np.asarray(x, np.float32)
    w_qkv = np.asarray(w_qkv, np.float32)
    w_proj = np.asarray(w_proj, np.float32)
    b_proj = np.asarray(b_proj, np.float32)
    B = x.shape[0]
    assert x.shape == (8, NTOK, C), x.shape

    mt = np.float16
    wqkT = np.ascontiguousarray(w_qkv[:2 * C].T.astype(mt))
    wvT = np.ascontiguousarray(w_qkv[2 * C:].T.astype(mt))
    wpT = np.ascontiguousarray(w_proj.T.astype(mt))
    bp = np.ascontiguousarray(b_proj.reshape(C, 1))
    sel = np.zeros((128, 768), np.float16)
    for p in range(6):
        for hi in range(2):
            sel[2 * p + hi, 128 * p + 64 * hi:128 * p + 64 * hi + 64] = 1.0
    in_maps = []
    for b in range(B):
        xT = np.zeros((C, T), mt)
        xT[:, :NTOK] = x[b].T.astype(mt)
        in_maps.append({"xT": xT, "wqkT": wqkT, "wvT": wvT, "wpT": wpT, "bp": bp,
                        "sel": sel})
    return in_maps


def kernel(x, w_qkv, w_proj, b_proj):
    B = np.asarray(x).shape[0]
    in_maps = prep_in_maps(x, w_qkv, w_proj, b_proj, _MODE)

    if _MODE not in _NC_CACHE:
        _NC_CACHE[_MODE] = build(matmul_dtype=_MODE)
    nc = _NC_CACHE[_MODE]
    from concourse import bass_utils
    res = bass_utils.run_bass_kernel_spmd(nc, in_maps, core_ids=list(range(B)),
                                          trace=False)
    y = np.stack([res.results[b]["yT"][:, :NTOK].T for b in range(B)])
    return np.ascontiguousarray(y.astype(np.float32))
